# revision 29
# baseline (speedup 1.0000x reference)
"""Chamfer loss kernel for Trainium2 (8 NeuronCores, batch-parallel).

Problem: preds [8, 4096, 3] f32, gts [8, 4096, 3] f32.
  P[b,n,m] = ||gts[b,n] - preds[b,m]||^2  (expanded form)
  loss = sum_{b,m} min_n P[b,n,m] + sum_{b,n} min_m P[b,n,m]

Sharding: one batch per NeuronCore (data parallel over B=8).

Device algorithm (per core, one batch):
  Host augments points to 5-dim vectors so a single K=5 matmul emits
  squared distances directly into PSUM:
      a_n = [-2*x_n, ||x_n||^2, 1]   (x = gts row)
      b_m = [ y_m,   1, ||y_m||^2]   (y = preds row)
      a_n . b_m = ||x_n - y_m||^2  (same expanded form as the reference)

  Precision/speed: plain fp32 matmuls stream at 1/4 rate (and split 2x
  in codegen); fp32r is full rate but ~bf16 precision, which breaks the
  cancellation in the expanded form. So operands are split hi/lo into
  fp16 halves and each distance tile is TWO full-rate fp16 matmuls
  accumulated in fp32 PSUM:
      term1 (K=5):  a_hi . b_hi
      term2 (K=10): [a_hi; a_lo] . [b_lo; b_hi] = a_hi.b_lo + a_lo.b_hi
  (the lo.lo term ~1e-5 is dropped).

  K<=10 uses a sliver of the 128 PE rows, so each matmul wave is packed
  4x into the PE's 32-row tiles (tile_position (0,0)/(32,0)/(64,0)/
  (96,0)). The four concurrent matmuls compute four 512-column chunks
  of the SAME row-tile, one per PSUM bank, so a [128, 2048] PSUM
  generation holds half of one row-tile's distance row. Row-group q
  reads operands from SBUF partitions 32q.., so the host replicates the
  fp16 operand block at partition offsets 0/32/64/96.

  VectorE tensor_reduce(min) consumes each [128, 2048] PSUM generation
  along the free axis into half-row mins; a second tiny reduce folds the
  two halves into row mins and a reduce_sum folds those into
  per-partition sums [128, 1]. (The reduction is the wall: VectorE
  streams 1 elem/lane/cycle at 0.96 GHz and is the only engine that can
  both read PSUM and take a min — TENSOR_TENSOR_REDUCE and custom-DVE
  encodings, which could pair a PSUM and an SBUF stream for 2x, are
  rejected by this walrus build with "ISA wrong length".)
  Pass A: rows = n (gts) -> loss_2 terms; pass B: rows = m (preds) ->
  loss_1 terms.
Host sums the 8x128 partial sums (the gather/unshard step).
"""

import sys

import numpy as np

sys.path.insert(0, "/opt/trn_rl_repo")

B = 8
N = 4096  # points per cloud (both preds and gts)
D5 = 5  # augmented dim
P = 128  # partitions
N_CORES = 8
NBANK = 4  # psum banks per generation = packed chunks
NTILES = N // P  # 32 row-tiles per pass


def _build_kernel_body(ctx, tc, out_ap, ab_ap):
    import concourse.bass as bass
    from concourse import mybir

    nc = tc.nc
    f16 = mybir.dt.float16
    f32 = mybir.dt.float32

    const = ctx.enter_context(tc.tile_pool(name="const", bufs=1))
    psum = ctx.enter_context(tc.tile_pool(name="psum", bufs=2, space="PSUM"))
    stage_pool = ctx.enter_context(tc.tile_pool(name="stage", bufs=8))

    # Four operand blocks, each [10, N] fp16 per 32-partition group:
    #   block 0: at10  = [at_hi; at_lo]    block 1: bt10  = [bt_hi; bt_lo]
    #   block 2: at10s = [at_lo; at_hi]    block 3: bt10s = [bt_lo; bt_hi]
    # replicated on partition groups 0/32/64/96 (one per PE row-tile).
    # One DMA -> a single DMA semaphore for the first matmuls to wait on
    # (the Matmult/LDWEIGHTS struct has one sync-wait slot).
    ab_sb = const.tile([P, 4 * N], f16)
    nc.sync.dma_start(out=ab_sb[:], in_=ab_ap[:])

    def blk(q, which, rows, cols):
        return ab_sb[
            32 * q + rows.start : 32 * q + rows.stop,
            which * N + cols.start : which * N + cols.stop,
        ]

    AT10, BT10, AT10S, BT10S = 0, 1, 2, 3
    R5, R10 = slice(0, 5), slice(0, 10)

    rowmins = const.tile([P, 2 * NTILES], f32)

    # Half-row mins: [:, k, h] = min over chunks 4h..4h+3 of row-tile k.
    halfmins = const.tile([P, 2 * NTILES, 2], f32)

    # Constant source for the ScalarE slot-claim write (see below).
    claim_src = const.tile([P, 1], f16)
    nc.vector.memset(claim_src[:], 0.0)

    for pass_idx, (wb, rb, rbs) in enumerate(((AT10, BT10, BT10S), (BT10, AT10, AT10S))):
        for i in range(NTILES):
            icols = slice(i * P, (i + 1) * P)
            for half in range(2):  # chunks 0-3, then 4-7
                ps = psum.tile([P, NBANK * 512], f32, tag="ps")
                for w_rows, r_which, r_rows, start, stop in (
                    (R5, rb, R5, True, False),
                    (R10, rbs, R10, False, True),
                ):
                    for q in range(NBANK):
                        c = half * NBANK + q
                        nc.tensor.matmul(
                            ps[:, q * 512 : (q + 1) * 512],
                            blk(q, wb, w_rows, icols),
                            blk(q, r_which, r_rows, slice(c * 512, (c + 1) * 512)),
                            start=start,
                            stop=stop,
                            tile_position=(32 * q, 0),
                        )
                k = pass_idx * NTILES + i
                g_idx = k * 2 + half
                if g_idx % 8 == 0:
                    # Direct path: VectorE min-reduce from PSUM (1x rate).
                    nc.vector.tensor_reduce(
                        out=halfmins[:, k, half : half + 1],
                        in_=ps[:],
                        axis=mybir.AxisListType.X,
                        op=mybir.AluOpType.min,
                    )
                else:
                    # Fast path (7 of 8 generations): ScalarE converts
                    # PSUM->SBUF fp16 (rounding is monotone, so mins are
                    # preserved to ~1.5e-5); VectorE then folds pairwise
                    # with fp16 tensor_tensor(min), which runs at the
                    # 2x_1P perf mode (2 elems/lane/cycle) - ~1.5us/gen
                    # instead of 2.26us, with the copy on the idle ScalarE.
                    st = stage_pool.tile([P, NBANK * 512], f16, tag="st")
                    # The Activation struct also takes only one sync wait,
                    # but the copy needs two (PSUM ready + stage slot free).
                    # Split them: a 1-element claim write carries the
                    # slot-release wait; ScalarE's in-order execution then
                    # lets the full copy carry only the PSUM-ready wait.
                    nc.scalar.copy(st[:, 0:1], claim_src[:])
                    nc.scalar.copy(st[:], ps[:])
                    for w in (1024, 512, 256):
                        nc.vector.tensor_tensor(
                            out=st[:, 0:w],
                            in0=st[:, 0:w],
                            in1=st[:, w : 2 * w],
                            op=mybir.AluOpType.min,
                        )
                    nc.vector.tensor_reduce(
                        out=halfmins[:, k, half : half + 1],
                        in_=st[:, 0:256],
                        axis=mybir.AxisListType.X,
                        op=mybir.AluOpType.min,
                    )

    # Row min per row-tile = min over the two half-row mins.
    nc.vector.tensor_reduce(
        out=rowmins[:],
        in_=halfmins[:],
        axis=mybir.AxisListType.X,
        op=mybir.AluOpType.min,
    )
    # Per-partition sum of all row mins (both passes).
    sums = const.tile([P, 1], f32)
    nc.vector.tensor_reduce(
        out=sums[:], in_=rowmins[:], axis=mybir.AxisListType.X, op=mybir.AluOpType.add
    )
    nc.sync.dma_start(out=out_ap[:], in_=sums[:])


def _build_nc():
    from contextlib import ExitStack

    import concourse.bass as bass
    import concourse.tile as tile
    from concourse import mybir

    nc = bass.Bass("TRN2", target_bir_lowering=False, debug=False)
    ab = nc.dram_tensor(
        "ab", [P, 4 * N], mybir.dt.float16, kind="ExternalInput"
    ).ap()
    out = nc.dram_tensor("out", [P, 1], mybir.dt.float32, kind="ExternalOutput").ap()
    with tile.TileContext(nc) as tc, ExitStack() as ctx:
        _build_kernel_body(ctx, tc, out, ab)
    _fix_sync_waits(nc)
    return nc


def _fix_sync_waits(nc):
    """Work around walrus's one-sync-wait-per-struct codegen limits.

    1. Drop Matmult waits on the PE's own completion semaphore. Tile emits
       a PE-self wait to guard PSUM write-after-write across pool-slot
       generations, but the PE drains matmuls strictly in order
       (pc-monotone completion), so a PE instruction's write never
       overtakes an earlier PE instruction's write — the self-wait is
       redundant. The cross-engine wait (the previous slot generation's
       reader: VectorE reduce or ScalarE copy) is load-bearing and kept.
    2. Reduce the kernel-tail Drain's waits to just the output-DMA
       semaphore. In this kernel's dependency chain the output DMA waits
       on the final VectorE fold, which waits on every reduce and
       therefore on all PE work and the input DMA — so output-DMA
       completion transitively implies every other wait.
    """
    out_sems = set()
    for fn in nc.m.functions:
        for blk in fn.blocks:
            for ins in blk.instructions:
                if type(ins).__name__ != "InstDMACopy":
                    continue
                if any(getattr(o, "memref", None) == "out" for o in ins.outs):
                    for u in ins.sync_info.on_update:
                        out_sems.add(u.ant_name)
    assert out_sems, "output DMA not found"

    n_multi = 0
    for fn in nc.m.functions:
        for blk in fn.blocks:
            for ins in blk.instructions:
                tn = type(ins).__name__
                si = ins.sync_info
                if si is None:
                    continue
                if tn in ("InstMatmult", "InstActivation"):
                    # Engines execute and complete their own queues in
                    # order (PE pc-monotone, ScalarE strict FIFO), so a
                    # wait on the instruction's own engine semaphore is
                    # redundant; walrus only gives these structs one
                    # sync-wait slot.
                    self_pfx = "PE_" if tn == "InstMatmult" else "Activation_"
                    waits = list(si.on_wait)
                    if any(
                        w.ant_name and w.ant_name.startswith(self_pfx)
                        for w in waits
                    ):
                        si.on_wait = [
                            w
                            for w in waits
                            if not (w.ant_name and w.ant_name.startswith(self_pfx))
                        ]
                        ins.sync_info = si
                    if len(ins.sync_info.on_wait) > 1:
                        n_multi += 1
                elif tn == "InstDrain" and len(si.on_wait) > 1:
                    keep = [w for w in si.on_wait if w.ant_name in out_sems]
                    assert keep, (
                        f"tail drain {ins.name} lacks an output-DMA sem wait: "
                        f"{[(w.ant_name, w.wait_value) for w in si.on_wait]}"
                    )
                    si.on_wait = keep
                    ins.sync_info = si
    assert n_multi == 0, f"{n_multi} Matmult/Activation still carry >1 sync wait"


_NC_CACHE = {}


def _get_nc():
    if "nc" not in _NC_CACHE:
        _NC_CACHE["nc"] = _build_nc()
    return _NC_CACHE["nc"]


def _split_f16(a):
    """Split fp32 array into (hi, lo) fp16 halves with a ~= hi + lo."""
    hi = a.astype(np.float16)
    lo = (a - hi.astype(np.float32)).astype(np.float16)
    return hi, lo


def _make_in_maps(preds, gts):
    preds = np.ascontiguousarray(np.asarray(preds, dtype=np.float32))
    gts = np.ascontiguousarray(np.asarray(gts, dtype=np.float32))
    in_maps = []
    for b in range(B):
        x = gts[b]  # [N, 3]
        y = preds[b]  # [N, 3]
        rx = np.sum(x * x, axis=-1)  # [N]
        ry = np.sum(y * y, axis=-1)  # [N]
        at = np.empty((D5, N), np.float32)
        at[0:3] = (-2.0 * x).T
        at[3] = rx
        at[4] = 1.0
        bt = np.empty((D5, N), np.float32)
        bt[0:3] = y.T
        bt[3] = 1.0
        bt[4] = ry
        at_hi, at_lo = _split_f16(at)
        bt_hi, bt_lo = _split_f16(bt)
        at10 = np.concatenate([at_hi, at_lo], axis=0)  # [10, N]
        bt10 = np.concatenate([bt_hi, bt_lo], axis=0)
        at10s = np.concatenate([at_lo, at_hi], axis=0)
        bt10s = np.concatenate([bt_lo, bt_hi], axis=0)
        block = np.concatenate([at10, bt10, at10s, bt10s], axis=1)  # [10, 4N]
        ab = np.zeros((P, 4 * N), np.float16)
        for q in range(NBANK):  # replicate for each PE row-tile group
            ab[32 * q : 32 * q + 2 * D5] = block
        in_maps.append({"ab": ab})
    return in_maps


def run_device(preds, gts, **spmd_kwargs):
    """Run the on-device kernel; returns (per-core [128,1] partials, raw results)."""
    from concourse.bass_utils import run_bass_kernel_spmd

    nc = _get_nc()
    in_maps = _make_in_maps(preds, gts)
    res = run_bass_kernel_spmd(nc, in_maps, list(range(N_CORES)), **spmd_kwargs)
    partials = [np.asarray(r["out"]) for r in res.results]
    return partials, res


def kernel(preds, gts):
    partials, _ = run_device(preds, gts)
    total = np.sum(np.stack(partials, 0), dtype=np.float32)
    return np.asarray(total, dtype=np.float32)
